# revision 4
# baseline (speedup 1.0000x reference)
"""Two-layer GCN encoder on 8 Trainium2 NeuronCores (Bass/Tile).

Math (per layer, PyG GCNConv):
    deg[d]  = |{edges s->d}| + 1 (self loop)        [graph structure]
    dinv    = deg ** -0.5
    hs      = (dinv * x) @ W                        [= dinv * (x @ W)]
    agg[d]  = sum_{s in N(d)} hs[s] + hs[d]
    h       = relu(dinv * agg + b)                  [b == 0 here]
    out     = concat([h1, h2], axis=1)

Sharding: dst nodes are split evenly across the 8 cores.  Each core
computes hs for its own node shard (dense matmul), the shards are
AllGather'ed in four quarter-shard chunks (chunk k == int16 gather
window k, 25600 rows) into a replicated hs_full table in DRAM, and each
core pulls hs_full[src] for the non-self-loop edges pointing into its
shard with batched gather DMA (dma_gather, int16 indices).  The four
windows' gather calls go to the four SWDGE queues so descriptor
generation for all four windows runs concurrently (descriptor
generation, not DMA bandwidth, is the bottleneck of this kernel).

Messages for one (span of 7 dst blocks, window) pair are packed
contiguously into 128-edge tiles sorted by dst block; per-core padding
is trailing (idx -1) so it generates no descriptors.  A 0/1 selection
matrix per (dst block, tile) is built with per-tile tensor_scalar
is_equal ops (fp16 iota vs per-partition dst-slot scalar; fp16 keeps
the DVE in its packed fast mode and represents integers up to 2048
exactly) and routes each tile through one PE matmul that segment-sums
messages into a PSUM accumulator per dst block.  The self-loop
contribution comes from a resident SBUF copy of the core's own hs via
an identity matmul (start=True).  Postprocessing is fused into
scalar-engine activations: h = relu(dinv * agg), and the layer-2 input
s2 = dinv * h = relu(dinv^2 * agg).

Layer transition is fully pipelined: per span, s2 is stored, transposed
back into the (shared) xT tile with dma_start_transpose, and the span's
layer-2 hs matmuls run immediately.  Layer 2's four AllGather chunks
are emitted inside the layer-1 span loop, each pinned behind a span
whose gather dispatch time is provably later than the chunk's hs2
stores, so every chunk's transfer overlaps layer-1 aggregation instead
of serializing after it (the Pool stream is in-order: a collective
placed too early would stall gather dispatch until its inputs exist).
Only chunk 3 (the last quarter, produced by the final spans) lands
after layer 1; layer 2's first spans issue their window-0..2 gathers
ahead of it so the window-3 queue restarts with minimal idle.

Host-side work is limited to graph preprocessing: degree counts, edge
sorting, index layout, dtype casts.  All O(E*F) and O(N*F*F) floating
point work runs on the NeuronCores.
"""

import os

import numpy as np

from concourse import bacc, bass, mybir
import concourse.tile as tile
from concourse.bass_utils import run_bass_kernel_spmd
from concourse.tile_rust import add_dep_helper
from concourse.library_config import mlp

FP16 = np.float16
F32 = mybir.dt.float32
F16 = mybir.dt.float16
I16 = mybir.dt.int16

P = 128        # partitions / feature dim / edges per tile
SPAN = 7       # dst blocks per gather span
N_NODES = 100000
N_EDGES = 1600000
N_CORES = 8
FEAT = 128

NPC = N_NODES // N_CORES          # nodes per core (12500)
NBLK = 100                        # 128-node blocks per core (2 pad blocks)
NPCP = NBLK * P                   # padded nodes per core (12800)
NN = N_CORES * NPCP               # rows of the allgathered hs table (102400)
NWIN = 4
QROWS = NPCP // NWIN              # rows per core per AllGather chunk (3200)
QBLK = NBLK // NWIN               # blocks per core per chunk (25)
WROWS = NN // NWIN                # int16 gather window (25600 < 32768)
NBLK_AGG = -(-NPC // P)           # blocks with real dst nodes (98)
NSPAN = -(-NBLK_AGG // SPAN)      # aggregation spans per core (14)


class Cfg:  # retained so test.py's K.run(K.CFG, ...) keeps working
    pass


CFG = Cfg()


def _ceil(a, b):
    return -(-a // b)


# ---------------------------------------------------------------------------
# Host-side graph preprocessing (indices only, plus dtype casts)
# ---------------------------------------------------------------------------

def prep_inputs(x, edge_index, W1, b1, W2, b2):
    x = np.asarray(x, dtype=np.float32)
    src = np.asarray(edge_index[0], dtype=np.int64)
    dst = np.asarray(edge_index[1], dtype=np.int64)

    deg = (np.bincount(dst, minlength=N_NODES) + 1).astype(np.float64)
    dinv = (1.0 / np.sqrt(deg)).astype(np.float32)

    # table row of node v: shards padded to NPCP, split into quarter-shard
    # AllGather chunks: chunk q holds [core0 quarter_q, core1 quarter_q, ...]
    core_of = src // NPC
    loc = src % NPC
    q = loc // QROWS
    table_row = q * WROWS + core_of * QROWS + (loc - q * QROWS)

    core_of_dst = dst // NPC

    ncall = NSPAN * NWIN
    per_core = []
    cnts = np.zeros((N_CORES, ncall), dtype=np.int64)
    # per-core cumulative message count by (span, window, block-within-span)
    cumh = np.zeros((N_CORES, ncall, SPAN + 1), dtype=np.int64)
    for c in range(N_CORES):
        m = core_of_dst == c
        srows = table_row[m]
        dloc = dst[m] - c * NPC
        bg = dloc >> 7
        s = bg // SPAN
        w = srows // WROWS
        key = s * NWIN + w
        order = np.lexsort((bg, key))
        srows, key, dloc, bg = srows[order], key[order], dloc[order], bg[order]
        cnts[c] = np.bincount(key, minlength=ncall)
        bin_sb = np.bincount(key * SPAN + (bg % SPAN),
                             minlength=ncall * SPAN).reshape(ncall, SPAN)
        cumh[c, :, 1:] = np.cumsum(bin_sb, axis=1)
        per_core.append((srows, key, dloc))

    # tiles per call: max over cores -> identical program on every core
    T_call = _ceil(cnts.max(axis=0), P)  # [ncall]
    gt0 = np.zeros(ncall + 1, dtype=np.int64)
    gt0[1:] = np.cumsum(T_call)
    TT = int(gt0[-1])

    # conservative per-(block, window) tile ranges shared by all cores
    # ranges[b][w] = (t0, t1) inclusive, or None
    ranges = [[None] * NWIN for _ in range(NBLK_AGG)]
    for s in range(NSPAN):
        b0 = s * SPAN
        for w in range(NWIN):
            call = s * NWIN + w
            for k in range(min(SPAN, NBLK_AGG - b0)):
                lo = int(cumh[:, call, k].min())
                hi = int(cumh[:, call, k + 1].max())
                if hi > lo:
                    ranges[b0 + k][w] = (int(gt0[call]) + lo // P,
                                         int(gt0[call]) + _ceil(hi, P) - 1)

    in_maps = []
    for c in range(N_CORES):
        srows, key, dloc = per_core[c]
        start = np.concatenate([[0], np.cumsum(cnts[c])[:-1]])
        pos = np.arange(len(key)) - start[key]
        gtile = gt0[key] + (pos >> 7)
        gpart = pos & 127

        # pad slots gather row 0 (harmless) and carry dst slot -1 (masked by
        # the is_equal selection matrix); avoids the negative-index strip path
        V = np.zeros((TT, P), np.int64)          # window-local source row
        D = np.full((TT, P), -1.0, np.float32)   # span-local dst slot
        V[gtile, gpart] = srows % WROWS
        D[gtile, gpart] = dloc - (key // NWIN) * (SPAN * P)

        # idx16 layout: per call the columns [8*gt0, 8*gt1); msg j (t-major)
        # lives at [16g + (j%16), gt0*8 + j//16], replicated to 128 partitions
        idx16 = np.zeros((P, TT * 8), np.int16)
        for call in range(ncall):
            a, b = int(gt0[call]), int(gt0[call + 1])
            if b == a:
                continue
            v = V[a:b, :].reshape(-1)
            blockv = v.reshape(-1, 16).T.astype(np.int16)
            idx16[:, a * 8:b * 8] = np.tile(blockv, (8, 1))

        xs = x[c * NPC:(c + 1) * NPC] * dinv[c * NPC:(c + 1) * NPC, None]
        xT = np.zeros((P, NPCP), np.float32)
        xT[:, :NPC] = xs.T
        dv = np.zeros(NPCP, np.float32)
        dv[:NPC] = dinv[c * NPC:(c + 1) * NPC]
        dinvT = np.ascontiguousarray(dv.reshape(NBLK, P).T)

        iot7 = np.broadcast_to(np.arange(SPAN * P, dtype=np.float32),
                               (P, SPAN * P)).copy()

        in_maps.append(
            {
                "xT": xT.astype(FP16),
                "idx16": idx16,
                "dsel": np.ascontiguousarray(D.T).astype(np.float32),
                "dinvT": dinvT,
                "dinv2T": dinvT * dinvT,
                "w1": np.asarray(W1, np.float32).astype(FP16),
                "w2": np.asarray(W2, np.float32).astype(FP16),
                "iot7": iot7.astype(FP16),
                "ident": np.eye(P, dtype=np.float32).astype(FP16),
            }
        )
    return in_maps, T_call, ranges


# ---------------------------------------------------------------------------
# Device program
# ---------------------------------------------------------------------------

def build_program(T_call, ranges):
    n_f = FEAT
    gt0 = np.zeros(len(T_call) + 1, dtype=np.int64)
    gt0[1:] = np.cumsum(T_call)
    TT = int(gt0[-1])

    nc = bacc.Bacc("TRN2", target_bir_lowering=False, debug=False,
                   num_devices=N_CORES, num_swdge_queues=4)

    xT_d = nc.dram_tensor("xT", [P, NPCP], F16, kind="ExternalInput")
    idx16_d = nc.dram_tensor("idx16", [P, TT * 8], I16, kind="ExternalInput")
    dsel_d = nc.dram_tensor("dsel", [P, TT], F32, kind="ExternalInput")
    dinvT_d = nc.dram_tensor("dinvT", [P, NBLK], F32, kind="ExternalInput")
    dinv2T_d = nc.dram_tensor("dinv2T", [P, NBLK], F32, kind="ExternalInput")
    w_d = [nc.dram_tensor("w1", [n_f, n_f], F16, kind="ExternalInput"),
           nc.dram_tensor("w2", [n_f, n_f], F16, kind="ExternalInput")]
    iot7_d = nc.dram_tensor("iot7", [P, SPAN * P], F16, kind="ExternalInput")
    ident_d = nc.dram_tensor("ident", [P, P], F16, kind="ExternalInput")
    out_d = nc.dram_tensor("out", [NPC, 2 * n_f], F32, kind="ExternalOutput")

    s2_sh = nc.dram_tensor("s2sh", [NPCP, n_f], F16)
    hs_sh = [nc.dram_tensor(f"hs{L}sh", [NPCP, n_f], F16) for L in (1, 2)]
    hs_full = [nc.dram_tensor(f"hs{L}full", [NN, n_f], F16,
                              addr_space="Shared") for L in (1, 2)]
    groups = [list(range(N_CORES))]

    with tile.TileContext(nc) as tc:
        with (
            tc.tile_pool(name="const", bufs=1) as cpool,
            tc.tile_pool(name="big", bufs=1) as bigpool,
            tc.tile_pool(name="msg", bufs=3) as msgpool,
            tc.tile_pool(name="sel", bufs=10) as selpool,
            tc.tile_pool(name="post", bufs=6) as postpool,
            tc.tile_pool(name="psxw", bufs=2, space="PSUM") as psxw,
            tc.tile_pool(name="psag", bufs=6, space="PSUM") as psag,
        ):
            nc.gpsimd.load_library(mlp)
            w_t = []
            for L in (0, 1):
                wt = cpool.tile([n_f, n_f], F16, tag=f"w{L}", name=f"w{L}t")
                nc.sync.dma_start(out=wt[:], in_=w_d[L][:])
                w_t.append(wt)
            iot7_t = cpool.tile([P, SPAN * P], F16, tag="iot7", name="iot7_t")
            nc.sync.dma_start(out=iot7_t[:], in_=iot7_d[:])
            ident_t = cpool.tile([P, P], F16, tag="ident", name="ident_t")
            nc.sync.dma_start(out=ident_t[:], in_=ident_d[:])
            dinvT_t = cpool.tile([P, NBLK], F32, tag="dinvT", name="dinvT_t")
            nc.sync.dma_start(out=dinvT_t[:], in_=dinvT_d[:])
            dinv2T_t = cpool.tile([P, NBLK], F32, tag="dinv2T", name="dinv2T_t")
            nc.sync.dma_start(out=dinv2T_t[:], in_=dinv2T_d[:])

            # resident graph indices (shared by both layers)
            idx16_t = bigpool.tile([P, TT * 8], I16, tag="idx16", name="idx16_t")
            nc.sync.dma_start(out=idx16_t[:], in_=idx16_d[:])
            dsel_t = bigpool.tile([P, TT], F32, tag="dsel", name="dsel_t")
            nc.sync.dma_start(out=dsel_t[:], in_=dsel_d[:])

            # xT: layer-1 input, overwritten per span with transposed s2
            xT_t = bigpool.tile([P, NPCP], F16, tag="xT", name="xT_t")
            nc.sync.dma_start(out=xT_t[:], in_=xT_d[:])
            # resident own-shard hs (self-loop operand), overwritten per layer
            hso_t = bigpool.tile([P, NPCP], F16, tag="hso", name="hso_t")

            def xw_block(L, t):
                """hs_L[block t] = (xT[:, t].T @ W_L); store shard + SBUF copy.

                Layer-1 copies run on the (then idle) vector engine to shorten
                the startup ramp; layer-2 copies go to the scalar engine so
                they do not compete with the selection-matrix stream."""
                ps = psxw.tile([P, n_f], F32, tag="psxw", name="psxw_t")
                nc.tensor.matmul(out=ps[:], lhsT=xT_t[:, t * P:(t + 1) * P],
                                 rhs=w_t[L][:], start=True, stop=True)
                dst = hso_t[:, t * P:(t + 1) * P]
                if L == 0:
                    nc.vector.tensor_copy(out=dst, in_=ps[:])
                else:
                    nc.scalar.activation(out=dst, in_=ps[:],
                                         func=mybir.ActivationFunctionType.Copy)
                return nc.sync.dma_start(out=hs_sh[L][t * P:(t + 1) * P, :],
                                         in_=dst)

            def allgather_chunk(L, k, stores, pin=None):
                ag = nc.gpsimd.collective_compute(
                    "AllGather", mybir.AluOpType.bypass, replica_groups=groups,
                    ins=[hs_sh[L][k * QROWS:(k + 1) * QROWS, :]],
                    outs=[hs_full[L][k * WROWS:(k + 1) * WROWS, :]])
                for s in stores:
                    add_dep_helper(ag.ins, s.ins, reason="allgather after hs stores")
                if pin is not None:
                    add_dep_helper(ag.ins, pin.ins,
                                   reason="pin allgather into the gather stream")
                return ag

            glog = []  # gather instructions in emission order (for pinning)

            def span_gathers(L, s, ags, wins=range(NWIN), msg=None):
                """Issue window gather calls of span s (queue = window)."""
                t0 = int(gt0[s * NWIN])
                t1 = int(gt0[(s + 1) * NWIN])
                ts = t1 - t0
                if msg is None:
                    msg = msgpool.tile([P, ts, n_f], F16, tag="msg", name="msg_t")
                for w in wins:
                    a = int(gt0[s * NWIN + w])
                    b = int(gt0[s * NWIN + w + 1])
                    if b == a:
                        continue
                    nidx = (b - a) * P
                    g = nc.gpsimd.dma_gather(
                        msg[:, a - t0:b - t0, :],
                        hs_full[L][(w * WROWS):(w * WROWS + WROWS), :],
                        idx16_t[:, a * 8:b * 8],
                        nidx, nidx, n_f, single_packet=False, queue_num=w)
                    add_dep_helper(g.ins, ags[w].ins,
                                   reason="gather after allgather chunk")
                    glog.append(g)
                return msg, t0

            def span_agg(L, s, msg, t0, s2_stores):
                """Segment-sum + postprocess the 7 blocks of span s."""
                b0 = s * SPAN
                for k in range(min(SPAN, NBLK_AGG - b0)):
                    b = b0 + k
                    rlist = [ranges[b][w] for w in range(NWIN)
                             if ranges[b][w] is not None]
                    ps = psag.tile([P, n_f], F32, tag="psag", name="psag_t")
                    nmm = sum(r1 - r0 + 1 for r0, r1 in rlist)
                    nc.tensor.matmul(out=ps[:], lhsT=ident_t[:],
                                     rhs=hso_t[:, b * P:(b + 1) * P],
                                     start=True, stop=(nmm == 0))
                    j = 0
                    for r0, r1 in rlist:
                        rn = r1 - r0 + 1
                        sel = selpool.tile([P, rn, P], F16, tag="sel",
                                           name="sel_t")
                        for t in range(rn):
                            nc.vector.tensor_scalar(
                                out=sel[:, t, :],
                                in0=iot7_t[:, k * P:(k + 1) * P],
                                scalar1=dsel_t[:, r0 + t:r0 + t + 1],
                                scalar2=None,
                                op0=mybir.AluOpType.is_equal)
                        for t in range(rn):
                            nc.tensor.matmul(out=ps[:],
                                             lhsT=sel[:, t, :],
                                             rhs=msg[:, r0 + t - t0, :],
                                             start=False,
                                             stop=(j == nmm - 1))
                            j += 1
                    # h = relu(dinv * agg); s2 = dinv * h = relu(dinv^2 * agg)
                    h_t = postpool.tile([P, n_f], F32, tag="hrelu",
                                        name="hrelu_t")
                    nc.scalar.activation(out=h_t[:], in_=ps[:],
                                         func=mybir.ActivationFunctionType.Relu,
                                         scale=dinvT_t[:, b:b + 1])
                    rows = min(P, NPC - b * P)
                    nc.scalar.dma_start(
                        out=out_d[b * P:b * P + rows, L * n_f:(L + 1) * n_f],
                        in_=h_t[:rows, :])
                    if L == 0:
                        s2_t = postpool.tile([P, n_f], F16, tag="s2",
                                             name="s2_t")
                        nc.scalar.activation(
                            out=s2_t[:], in_=ps[:],
                            func=mybir.ActivationFunctionType.Relu,
                            scale=dinv2T_t[:, b:b + 1])
                        s2_stores.append(
                            nc.sync.dma_start(out=s2_sh[b * P:(b + 1) * P, :],
                                              in_=s2_t[:]))

            hs2_stores = []

            def span_xw2(s, s2_stores):
                """Transpose span s's s2 back into xT and run its hs2 matmuls."""
                b0, b1 = s * SPAN, min((s + 1) * SPAN, NBLK_AGG)
                tr = nc.sync.dma_start_transpose(
                    out=xT_t[:, b0 * P:b1 * P],
                    in_=s2_sh[b0 * P:b1 * P, :])
                for st in s2_stores:
                    add_dep_helper(tr.ins, st.ins, reason="transpose after s2")
                for t in range(b0, b1):
                    hs2_stores.append(xw_block(1, t))

            # ---- layer 1 dense matmuls + four chunked AllGathers ----
            # Chunk k (= gather window k) is emitted right after its 25
            # blocks' stores so the first gathers start as early as possible.
            st1 = [xw_block(0, t) for t in range(NBLK)]
            ag1 = [allgather_chunk(0, k, st1[k * QBLK:(k + 1) * QBLK])
                   for k in range(NWIN)]

            # ---- layer 1 aggregation, with layer-2 xw pipelined per span ----
            # Layer-2 AllGather chunks 0-2 are pinned behind later layer-1
            # spans' gathers: by the time the Pool stream reaches each one,
            # its hs2 inputs are already stored, so it dispatches without
            # stalling the stream and its transfer overlaps layer-1 work.
            ag2 = [None] * NWIN
            pending = []  # (span, s2_stores) awaiting span_xw2
            for s in range(NSPAN):
                msg, t0 = span_gathers(0, s, ag1)
                if s == 9:
                    ag2[0] = allgather_chunk(1, 0, hs2_stores[:QBLK],
                                             pin=glog[-1])
                elif s == 11:
                    ag2[1] = allgather_chunk(1, 1, hs2_stores[QBLK:2 * QBLK],
                                             pin=glog[-1])
                elif s == 13:
                    ag2[2] = allgather_chunk(1, 2,
                                             hs2_stores[2 * QBLK:3 * QBLK],
                                             pin=glog[-1])
                s2st = []
                span_agg(0, s, msg, t0, s2st)
                pending.append((s, s2st))
                # run xw2 for the previous span (keeps PE from stalling on
                # the s2 DRAM round-trip)
                if len(pending) > 1:
                    ps, pst = pending.pop(0)
                    span_xw2(ps, pst)
            for ps, pst in pending:
                span_xw2(ps, pst)

            # ---- layer 2 aggregation ----
            # Chunk 3's inputs are the last spans' hs2, so it can only run
            # now; spans 0-1 issue windows 0-2 first so the window-3 queue is
            # the only one that waits for it, then window-3 catches up while
            # spans 2+ proceed normally.
            m0, t00 = span_gathers(1, 0, ag2, wins=(0, 1, 2))
            m1, t01 = span_gathers(1, 1, ag2, wins=(0, 1, 2))
            ag2[3] = allgather_chunk(1, 3, hs2_stores[3 * QBLK:],
                                     pin=glog[-1])
            span_gathers(1, 0, ag2, wins=(3,), msg=m0)
            span_gathers(1, 1, ag2, wins=(3,), msg=m1)
            span_agg(1, 0, m0, t00, [])
            span_agg(1, 1, m1, t01, [])
            for s in range(2, NSPAN):
                msg, t0 = span_gathers(1, s, ag2)
                span_agg(1, s, msg, t0, [])

    nc.compile()
    return nc


# ---------------------------------------------------------------------------
# Entry point
# ---------------------------------------------------------------------------

_CACHE: dict = {}


def _install_ntff_hook():
    """Wire the axon NTFF profiling hook that this image leaves unplugged.

    Harness-side instrumentation only; no-op when already present or
    when the pieces are missing."""
    try:
        from antenv.axon_hooks import get_axon_ntff_profile_hook  # noqa: F401
        return
    except ImportError:
        pass
    try:
        import sys
        import types

        if "/root/.axon_site" not in sys.path:
            sys.path.insert(0, "/root/.axon_site")
        from trn_agent_boot.trn_boot import _ntff_profile_via_ctypes

        hook = _ntff_profile_via_ctypes("/opt/axon/libaxon_pjrt.so")
        import antenv

        m = types.ModuleType("antenv.axon_hooks")
        m.get_axon_ntff_profile_hook = lambda: hook
        m.set_axon_ntff_profile_hook = lambda h: None
        sys.modules["antenv.axon_hooks"] = m
        antenv.axon_hooks = m
        import concourse.bass_utils as bu

        bu.upload_artifacts = lambda tmpdir: f"local:{tmpdir}"
    except Exception as e:  # degrade to no tracing
        print("ntff hook install failed:", e)


def run(cfg, inputs: dict, trace: bool = False):
    if trace:
        _install_ntff_hook()
    in_maps, T_call, ranges = prep_inputs(**inputs)
    key = (T_call.tobytes(), str(ranges))
    if key not in _CACHE:
        _CACHE[key] = build_program(T_call, ranges)
    nc = _CACHE[key]
    res = run_bass_kernel_spmd(nc, in_maps, list(range(N_CORES)), trace=trace)
    out = np.concatenate([res.results[c]["out"] for c in range(N_CORES)], axis=0)
    return out, res


def kernel(**inputs) -> np.ndarray:
    trace = bool(os.environ.get("BASS_TRACE"))
    try:
        out, _ = run(CFG, inputs, trace=trace)
    except Exception:
        # transient NRT / device hiccups happen rarely; one retry
        out, _ = run(CFG, inputs, trace=trace)
    return out


# revision 6
# speedup vs baseline: 1.1379x; 1.1379x over previous
"""Two-layer GCN encoder on 8 Trainium2 NeuronCores (Bass/Tile).

Math (per layer, PyG GCNConv):
    deg[d]  = |{edges s->d}| + 1 (self loop)        [graph structure]
    dinv    = deg ** -0.5
    hs      = (dinv * x) @ W                        [= dinv * (x @ W)]
    agg[d]  = sum_{s in N(d)} hs[s] + hs[d]
    h       = relu(dinv * agg + b)                  [b == 0 here]
    out     = concat([h1, h2], axis=1)

Sharding: dst nodes are split evenly across the 8 cores.  Each core
computes hs for its own node shard (dense matmul), the shards are
AllGather'ed in four quarter-shard chunks (chunk k == int16 gather
window k, 25600 rows) into a replicated hs_full table in DRAM, and each
core pulls hs_full[src] for the non-self-loop edges pointing into its
shard with batched gather DMA (dma_gather, int16 indices).  The four
windows' gather calls go to the four SWDGE queues so descriptor
generation for all four windows runs concurrently (descriptor
generation, not DMA bandwidth, is the bottleneck of this kernel).

Messages for one (span of 7 dst blocks, window) pair are packed
contiguously into 128-edge tiles sorted by dst block; per-core padding
is trailing (idx -1) so it generates no descriptors.  A 0/1 selection
matrix per (dst block, tile) is built with per-tile tensor_scalar
is_equal ops (fp16 iota vs per-partition dst-slot scalar; fp16 keeps
the DVE in its packed fast mode and represents integers up to 2048
exactly) and routes each tile through one PE matmul that segment-sums
messages into a PSUM accumulator per dst block.  The self-loop
contribution comes from a resident SBUF copy of the core's own hs via
an identity matmul (start=True).  Postprocessing is fused into
scalar-engine activations: h = relu(dinv * agg), and the layer-2 input
s2 = dinv * h = relu(dinv^2 * agg).

Layer transition is fully pipelined: per span, s2 is stored, transposed
back into the (shared) xT tile with dma_start_transpose, and the span's
layer-2 hs matmuls run immediately.  Layer 2's four AllGather chunks
are emitted inside the layer-1 span loop, each pinned behind a span
whose gather dispatch time is provably later than the chunk's hs2
stores, so every chunk's transfer overlaps layer-1 aggregation instead
of serializing after it (the Pool stream is in-order: a collective
placed too early would stall gather dispatch until its inputs exist).
Only chunk 3 (the last quarter, produced by the final spans) lands
after layer 1; layer 2's first spans issue their window-0..2 gathers
ahead of it so the window-3 queue restarts with minimal idle.

Host-side work is limited to graph preprocessing: degree counts, edge
sorting, index layout, dtype casts.  All O(E*F) and O(N*F*F) floating
point work runs on the NeuronCores.
"""

import os

import numpy as np

from concourse import bacc, bass, mybir
import concourse.tile as tile
from concourse.bass_utils import run_bass_kernel_spmd
from concourse.tile_rust import add_dep_helper
from concourse.library_config import mlp

FP16 = np.float16
F32 = mybir.dt.float32
F16 = mybir.dt.float16
I16 = mybir.dt.int16

P = 128        # partitions / feature dim / edges per tile
SPAN = 7       # dst blocks per gather span
N_NODES = 100000
N_EDGES = 1600000
N_CORES = 8
FEAT = 128

NPC = N_NODES // N_CORES          # nodes per core (12500)
NBLK = 100                        # 128-node blocks per core (2 pad blocks)
NPCP = NBLK * P                   # padded nodes per core (12800)
NN = N_CORES * NPCP               # rows of the allgathered hs table (102400)
NWIN = 4
QROWS = NPCP // NWIN              # rows per core per AllGather chunk (3200)
QBLK = NBLK // NWIN               # blocks per core per chunk (25)
WROWS = NN // NWIN                # int16 gather window (25600 < 32768)
NBLK_AGG = -(-NPC // P)           # blocks with real dst nodes (98)
NSPAN = -(-NBLK_AGG // SPAN)      # aggregation spans per core (14)


class Cfg:  # retained so test.py's K.run(K.CFG, ...) keeps working
    pass


CFG = Cfg()


def _ceil(a, b):
    return -(-a // b)


# ---------------------------------------------------------------------------
# Host-side graph preprocessing (indices only, plus dtype casts)
# ---------------------------------------------------------------------------

def prep_inputs(x, edge_index, W1, b1, W2, b2):
    x = np.asarray(x, dtype=np.float32)
    src = np.asarray(edge_index[0], dtype=np.int64)
    dst = np.asarray(edge_index[1], dtype=np.int64)

    deg = (np.bincount(dst, minlength=N_NODES) + 1).astype(np.float64)
    dinv = (1.0 / np.sqrt(deg)).astype(np.float32)

    # table row of node v: shards padded to NPCP, split into quarter-shard
    # AllGather chunks: chunk q holds [core0 quarter_q, core1 quarter_q, ...]
    core_of = src // NPC
    loc = src % NPC
    q = loc // QROWS
    table_row = q * WROWS + core_of * QROWS + (loc - q * QROWS)

    core_of_dst = dst // NPC

    ncall = NSPAN * NWIN
    per_core = []
    cnts = np.zeros((N_CORES, ncall), dtype=np.int64)
    # per-core cumulative message count by (span, window, block-within-span)
    cumh = np.zeros((N_CORES, ncall, SPAN + 1), dtype=np.int64)
    for c in range(N_CORES):
        m = core_of_dst == c
        srows = table_row[m]
        dloc = dst[m] - c * NPC
        bg = dloc >> 7
        s = bg // SPAN
        w = srows // WROWS
        key = s * NWIN + w
        order = np.lexsort((bg, key))
        srows, key, dloc, bg = srows[order], key[order], dloc[order], bg[order]
        cnts[c] = np.bincount(key, minlength=ncall)
        bin_sb = np.bincount(key * SPAN + (bg % SPAN),
                             minlength=ncall * SPAN).reshape(ncall, SPAN)
        cumh[c, :, 1:] = np.cumsum(bin_sb, axis=1)
        per_core.append((srows, key, dloc))

    # tiles per call: max over cores -> identical program on every core
    T_call = _ceil(cnts.max(axis=0), P)  # [ncall]
    gt0 = np.zeros(ncall + 1, dtype=np.int64)
    gt0[1:] = np.cumsum(T_call)
    TT = int(gt0[-1])

    # conservative per-(block, window) tile ranges shared by all cores
    # ranges[b][w] = (t0, t1) inclusive, or None
    ranges = [[None] * NWIN for _ in range(NBLK_AGG)]
    for s in range(NSPAN):
        b0 = s * SPAN
        for w in range(NWIN):
            call = s * NWIN + w
            for k in range(min(SPAN, NBLK_AGG - b0)):
                lo = int(cumh[:, call, k].min())
                hi = int(cumh[:, call, k + 1].max())
                if hi > lo:
                    ranges[b0 + k][w] = (int(gt0[call]) + lo // P,
                                         int(gt0[call]) + _ceil(hi, P) - 1)

    in_maps = []
    for c in range(N_CORES):
        srows, key, dloc = per_core[c]
        start = np.concatenate([[0], np.cumsum(cnts[c])[:-1]])
        pos = np.arange(len(key)) - start[key]
        gtile = gt0[key] + (pos >> 7)
        gpart = pos & 127

        # pad slots gather row 0 (harmless) and carry dst slot -1 (masked by
        # the is_equal selection matrix); avoids the negative-index strip path
        V = np.zeros((TT, P), np.int64)          # window-local source row
        D = np.full((TT, P), -1.0, np.float32)   # span-local dst slot
        V[gtile, gpart] = srows % WROWS
        D[gtile, gpart] = dloc - (key // NWIN) * (SPAN * P)

        # idx16 layout: per call the columns [8*gt0, 8*gt1); msg j (t-major)
        # lives at [16g + (j%16), gt0*8 + j//16], replicated to 128 partitions
        idx16 = np.zeros((P, TT * 8), np.int16)
        for call in range(ncall):
            a, b = int(gt0[call]), int(gt0[call + 1])
            if b == a:
                continue
            v = V[a:b, :].reshape(-1)
            blockv = v.reshape(-1, 16).T.astype(np.int16)
            idx16[:, a * 8:b * 8] = np.tile(blockv, (8, 1))

        xs = x[c * NPC:(c + 1) * NPC] * dinv[c * NPC:(c + 1) * NPC, None]
        xT = np.zeros((P, NPCP), np.float32)
        xT[:, :NPC] = xs.T
        dv = np.zeros(NPCP, np.float32)
        dv[:NPC] = dinv[c * NPC:(c + 1) * NPC]
        dinvT = np.ascontiguousarray(dv.reshape(NBLK, P).T)

        iot7 = np.broadcast_to(np.arange(SPAN * P, dtype=np.float32),
                               (P, SPAN * P)).copy()

        in_maps.append(
            {
                "xT": xT.astype(FP16),
                "idx16": idx16,
                "dsel": np.ascontiguousarray(D.T).astype(FP16),
                "dinvT": dinvT,
                "dinv2T": dinvT * dinvT,
                "w1": np.asarray(W1, np.float32).astype(FP16),
                "w2": np.asarray(W2, np.float32).astype(FP16),
                "iot7": iot7.astype(FP16),
                "ident": np.eye(P, dtype=np.float32).astype(FP16),
            }
        )
    return in_maps, T_call, ranges


# ---------------------------------------------------------------------------
# Device program
# ---------------------------------------------------------------------------

def build_program(T_call, ranges):
    n_f = FEAT
    gt0 = np.zeros(len(T_call) + 1, dtype=np.int64)
    gt0[1:] = np.cumsum(T_call)
    TT = int(gt0[-1])

    nc = bacc.Bacc("TRN2", target_bir_lowering=False, debug=False,
                   num_devices=N_CORES, num_swdge_queues=4)

    xT_d = nc.dram_tensor("xT", [P, NPCP], F16, kind="ExternalInput")
    idx16_d = nc.dram_tensor("idx16", [P, TT * 8], I16, kind="ExternalInput")
    dsel_d = nc.dram_tensor("dsel", [P, TT], F16, kind="ExternalInput")
    dinvT_d = nc.dram_tensor("dinvT", [P, NBLK], F32, kind="ExternalInput")
    dinv2T_d = nc.dram_tensor("dinv2T", [P, NBLK], F32, kind="ExternalInput")
    w_d = [nc.dram_tensor("w1", [n_f, n_f], F16, kind="ExternalInput"),
           nc.dram_tensor("w2", [n_f, n_f], F16, kind="ExternalInput")]
    iot7_d = nc.dram_tensor("iot7", [P, SPAN * P], F16, kind="ExternalInput")
    ident_d = nc.dram_tensor("ident", [P, P], F16, kind="ExternalInput")
    out_d = nc.dram_tensor("out", [NPC, 2 * n_f], F32, kind="ExternalOutput")

    s2_sh = nc.dram_tensor("s2sh", [NPCP, n_f], F16)
    hs_sh = [nc.dram_tensor(f"hs{L}sh", [NPCP, n_f], F16) for L in (1, 2)]
    hs_full = [nc.dram_tensor(f"hs{L}full", [NN, n_f], F16,
                              addr_space="Shared") for L in (1, 2)]
    groups = [list(range(N_CORES))]

    with tile.TileContext(nc) as tc:
        with (
            tc.tile_pool(name="const", bufs=1) as cpool,
            tc.tile_pool(name="big", bufs=1) as bigpool,
            tc.tile_pool(name="msg", bufs=3) as msgpool,
            tc.tile_pool(name="sel", bufs=10) as selpool,
            tc.tile_pool(name="post", bufs=6) as postpool,
            tc.tile_pool(name="psxw", bufs=2, space="PSUM") as psxw,
            tc.tile_pool(name="psag", bufs=6, space="PSUM") as psag,
        ):
            nc.gpsimd.load_library(mlp)
            w_t = []
            for L in (0, 1):
                wt = cpool.tile([n_f, n_f], F16, tag=f"w{L}", name=f"w{L}t")
                nc.sync.dma_start(out=wt[:], in_=w_d[L][:])
                w_t.append(wt)
            iot7_t = cpool.tile([P, SPAN * P], F16, tag="iot7", name="iot7_t")
            nc.sync.dma_start(out=iot7_t[:], in_=iot7_d[:])
            ident_t = cpool.tile([P, P], F16, tag="ident", name="ident_t")
            nc.sync.dma_start(out=ident_t[:], in_=ident_d[:])
            dinvT_t = cpool.tile([P, NBLK], F32, tag="dinvT", name="dinvT_t")
            nc.sync.dma_start(out=dinvT_t[:], in_=dinvT_d[:])
            dinv2T_t = cpool.tile([P, NBLK], F32, tag="dinv2T", name="dinv2T_t")
            nc.sync.dma_start(out=dinv2T_t[:], in_=dinv2T_d[:])

            # resident graph indices (shared by both layers)
            idx16_t = bigpool.tile([P, TT * 8], I16, tag="idx16", name="idx16_t")
            nc.sync.dma_start(out=idx16_t[:], in_=idx16_d[:])
            dsel_t = bigpool.tile([P, TT], F16, tag="dsel", name="dsel_t")
            nc.sync.dma_start(out=dsel_t[:], in_=dsel_d[:])

            # xT: layer-1 input, overwritten per span with transposed s2
            xT_t = bigpool.tile([P, NPCP], F16, tag="xT", name="xT_t")
            nc.sync.dma_start(out=xT_t[:], in_=xT_d[:])
            # resident own-shard hs (self-loop operand), overwritten per layer
            hso_t = bigpool.tile([P, NPCP], F16, tag="hso", name="hso_t")

            def xw_block(L, t):
                """hs_L[block t] = (xT[:, t].T @ W_L); store shard + SBUF copy.

                Layer-1 copies run on the (then idle) vector engine to shorten
                the startup ramp; layer-2 copies go to the scalar engine so
                they do not compete with the selection-matrix stream."""
                ps = psxw.tile([P, n_f], F32, tag="psxw", name="psxw_t")
                nc.tensor.matmul(out=ps[:], lhsT=xT_t[:, t * P:(t + 1) * P],
                                 rhs=w_t[L][:], start=True, stop=True)
                dst = hso_t[:, t * P:(t + 1) * P]
                if L == 0:
                    nc.vector.tensor_copy(out=dst, in_=ps[:])
                else:
                    nc.scalar.activation(out=dst, in_=ps[:],
                                         func=mybir.ActivationFunctionType.Copy)
                return nc.sync.dma_start(out=hs_sh[L][t * P:(t + 1) * P, :],
                                         in_=dst)

            def allgather_chunk(L, k, stores, pin=None):
                ag = nc.gpsimd.collective_compute(
                    "AllGather", mybir.AluOpType.bypass, replica_groups=groups,
                    ins=[hs_sh[L][k * QROWS:(k + 1) * QROWS, :]],
                    outs=[hs_full[L][k * WROWS:(k + 1) * WROWS, :]])
                for s in stores:
                    add_dep_helper(ag.ins, s.ins, reason="allgather after hs stores")
                if pin is not None:
                    add_dep_helper(ag.ins, pin.ins,
                                   reason="pin allgather into the gather stream")
                return ag

            glog = []  # gather instructions in emission order (for pinning)

            def span_gathers(L, s, ags, wins=range(NWIN), msg=None):
                """Issue window gather calls of span s (queue = window)."""
                t0 = int(gt0[s * NWIN])
                t1 = int(gt0[(s + 1) * NWIN])
                ts = t1 - t0
                if msg is None:
                    msg = msgpool.tile([P, ts, n_f], F16, tag="msg", name="msg_t")
                for w in wins:
                    a = int(gt0[s * NWIN + w])
                    b = int(gt0[s * NWIN + w + 1])
                    if b == a:
                        continue
                    nidx = (b - a) * P
                    g = nc.gpsimd.dma_gather(
                        msg[:, a - t0:b - t0, :],
                        hs_full[L][(w * WROWS):(w * WROWS + WROWS), :],
                        idx16_t[:, a * 8:b * 8],
                        nidx, nidx, n_f, single_packet=False, queue_num=w)
                    add_dep_helper(g.ins, ags[w].ins,
                                   reason="gather after allgather chunk")
                    glog.append(g)
                return msg, t0

            def span_agg(L, s, msg, t0, s2_stores):
                """Segment-sum + postprocess the 7 blocks of span s."""
                b0 = s * SPAN
                for k in range(min(SPAN, NBLK_AGG - b0)):
                    b = b0 + k
                    rlist = [ranges[b][w] for w in range(NWIN)
                             if ranges[b][w] is not None]
                    ps = psag.tile([P, n_f], F32, tag="psag", name="psag_t")
                    nmm = sum(r1 - r0 + 1 for r0, r1 in rlist)
                    nc.tensor.matmul(out=ps[:], lhsT=ident_t[:],
                                     rhs=hso_t[:, b * P:(b + 1) * P],
                                     start=True, stop=(nmm == 0))
                    j = 0
                    for r0, r1 in rlist:
                        rn = r1 - r0 + 1
                        sel = selpool.tile([P, rn, P], F16, tag="sel",
                                           name="sel_t")
                        nc.vector.tensor_tensor(
                            out=sel[:],
                            in0=iot7_t[:, None, k * P:(k + 1) * P]
                                .to_broadcast([P, rn, P]),
                            in1=dsel_t[:, r0:r1 + 1, None]
                                .to_broadcast([P, rn, P]),
                            op=mybir.AluOpType.is_equal)
                        for t in range(rn):
                            nc.tensor.matmul(out=ps[:],
                                             lhsT=sel[:, t, :],
                                             rhs=msg[:, r0 + t - t0, :],
                                             start=False,
                                             stop=(j == nmm - 1))
                            j += 1
                    # h = relu(dinv * agg); s2 = dinv * h = relu(dinv^2 * agg)
                    h_t = postpool.tile([P, n_f], F32, tag="hrelu",
                                        name="hrelu_t")
                    nc.scalar.activation(out=h_t[:], in_=ps[:],
                                         func=mybir.ActivationFunctionType.Relu,
                                         scale=dinvT_t[:, b:b + 1])
                    rows = min(P, NPC - b * P)
                    nc.scalar.dma_start(
                        out=out_d[b * P:b * P + rows, L * n_f:(L + 1) * n_f],
                        in_=h_t[:rows, :])
                    if L == 0:
                        s2_t = postpool.tile([P, n_f], F16, tag="s2",
                                             name="s2_t")
                        nc.scalar.activation(
                            out=s2_t[:], in_=ps[:],
                            func=mybir.ActivationFunctionType.Relu,
                            scale=dinv2T_t[:, b:b + 1])
                        s2_stores.append(
                            nc.sync.dma_start(out=s2_sh[b * P:(b + 1) * P, :],
                                              in_=s2_t[:]))

            hs2_stores = []

            def span_xw2(s, s2_stores):
                """Transpose span s's s2 back into xT and run its hs2 matmuls."""
                b0, b1 = s * SPAN, min((s + 1) * SPAN, NBLK_AGG)
                tr = nc.sync.dma_start_transpose(
                    out=xT_t[:, b0 * P:b1 * P],
                    in_=s2_sh[b0 * P:b1 * P, :])
                for st in s2_stores:
                    add_dep_helper(tr.ins, st.ins, reason="transpose after s2")
                for t in range(b0, b1):
                    hs2_stores.append(xw_block(1, t))

            # ---- layer 1 dense matmuls + four chunked AllGathers ----
            # Chunk k (= gather window k) is emitted right after its 25
            # blocks' stores so the first gathers start as early as possible.
            st1 = [xw_block(0, t) for t in range(NBLK)]
            ag1 = [allgather_chunk(0, k, st1[k * QBLK:(k + 1) * QBLK])
                   for k in range(NWIN)]

            # ---- layer 1 aggregation, with layer-2 xw pipelined per span ----
            # Layer-2 AllGather chunks 0-2 are pinned behind later layer-1
            # spans' gathers: by the time the Pool stream reaches each one,
            # its hs2 inputs are already stored, so it dispatches without
            # stalling the stream and its transfer overlaps layer-1 work.
            ag2 = [None] * NWIN
            pending = []  # (span, s2_stores) awaiting span_xw2
            for s in range(NSPAN):
                msg, t0 = span_gathers(0, s, ag1)
                if s == 9:
                    ag2[0] = allgather_chunk(1, 0, hs2_stores[:QBLK],
                                             pin=glog[-1])
                elif s == 11:
                    ag2[1] = allgather_chunk(1, 1, hs2_stores[QBLK:2 * QBLK],
                                             pin=glog[-1])
                elif s == 13:
                    ag2[2] = allgather_chunk(1, 2,
                                             hs2_stores[2 * QBLK:3 * QBLK],
                                             pin=glog[-1])
                s2st = []
                span_agg(0, s, msg, t0, s2st)
                pending.append((s, s2st))
                # run xw2 for the previous span (keeps PE from stalling on
                # the s2 DRAM round-trip)
                if len(pending) > 1:
                    ps, pst = pending.pop(0)
                    span_xw2(ps, pst)
            for ps, pst in pending:
                span_xw2(ps, pst)

            # ---- layer 2 aggregation ----
            # Chunk 3's inputs are the last spans' hs2, so it can only run
            # now; spans 0-1 issue windows 0-2 first so the window-3 queue is
            # the only one that waits for it, then window-3 catches up while
            # spans 2+ proceed normally.
            m0, t00 = span_gathers(1, 0, ag2, wins=(0, 1, 2))
            m1, t01 = span_gathers(1, 1, ag2, wins=(0, 1, 2))
            ag2[3] = allgather_chunk(1, 3, hs2_stores[3 * QBLK:],
                                     pin=glog[-1])
            span_gathers(1, 0, ag2, wins=(3,), msg=m0)
            span_gathers(1, 1, ag2, wins=(3,), msg=m1)
            span_agg(1, 0, m0, t00, [])
            span_agg(1, 1, m1, t01, [])
            for s in range(2, NSPAN):
                msg, t0 = span_gathers(1, s, ag2)
                span_agg(1, s, msg, t0, [])

    nc.compile()
    return nc


# ---------------------------------------------------------------------------
# Entry point
# ---------------------------------------------------------------------------

_CACHE: dict = {}


def _install_ntff_hook():
    """Wire the axon NTFF profiling hook that this image leaves unplugged.

    Harness-side instrumentation only; no-op when already present or
    when the pieces are missing."""
    try:
        from antenv.axon_hooks import get_axon_ntff_profile_hook  # noqa: F401
        return
    except ImportError:
        pass
    try:
        import sys
        import types

        if "/root/.axon_site" not in sys.path:
            sys.path.insert(0, "/root/.axon_site")
        from trn_agent_boot.trn_boot import _ntff_profile_via_ctypes

        hook = _ntff_profile_via_ctypes("/opt/axon/libaxon_pjrt.so")
        import antenv

        m = types.ModuleType("antenv.axon_hooks")
        m.get_axon_ntff_profile_hook = lambda: hook
        m.set_axon_ntff_profile_hook = lambda h: None
        sys.modules["antenv.axon_hooks"] = m
        antenv.axon_hooks = m
        import concourse.bass_utils as bu

        bu.upload_artifacts = lambda tmpdir: f"local:{tmpdir}"
    except Exception as e:  # degrade to no tracing
        print("ntff hook install failed:", e)


def run(cfg, inputs: dict, trace: bool = False):
    if trace:
        _install_ntff_hook()
    in_maps, T_call, ranges = prep_inputs(**inputs)
    key = (T_call.tobytes(), str(ranges))
    if key not in _CACHE:
        _CACHE[key] = build_program(T_call, ranges)
    nc = _CACHE[key]
    res = run_bass_kernel_spmd(nc, in_maps, list(range(N_CORES)), trace=trace)
    out = np.concatenate([res.results[c]["out"] for c in range(N_CORES)], axis=0)
    return out, res


def kernel(**inputs) -> np.ndarray:
    trace = bool(os.environ.get("BASS_TRACE"))
    try:
        out, _ = run(CFG, inputs, trace=trace)
    except Exception:
        # transient NRT / device hiccups happen rarely; one retry
        out, _ = run(CFG, inputs, trace=trace)
    return out
